# revision 15
# baseline (speedup 1.0000x reference)
"""Multi-head self-attention (shared q/k/v projection per head) + output
projection + LayerNorm, data-parallel over batch across 8 NeuronCores.

Shapes (hardcoded): B=8, S=512, E=768, H=12.
Each core handles one batch element b: full attention for all 12 heads,
the output projection, and the final LayerNorm. No collectives; the host
scatters x/mask per batch element and concatenates the 8 outputs.

v2: all big matmul operands in bf16 (weights converted host-side).
  - bf16 enables Fast Weight Load (LDWEIGHTS at 2x; fp32r disables FWL),
    halves weight DMA (56->28 MB/core), and makes PE transposes 1.0
    cycles/row instead of 1.5.
  - PSUM accumulation stays fp32; y accumulation across heads in SBUF
    fp32; LayerNorm in fp32. Measured rel err ~2e-3 (tolerance 2e-2).
  - LayerNorm uses DVE bn_stats/bn_aggr (one pass for mean+var) and is
    emitted inside head 11's y loop so it overlaps the last head's PE
    work instead of running as a serial tail.

Per-core dataflow:
  xT  = x^T                       (24 PE transposes, once)
  per head h:
    pT[e,s]   = Wh_h^T @ xT  + bh (36 MMs; bias applied in ACT psum->sbuf copy)
    scT[k,q]  = pT^T chunks @ pT  (24 MMs, scores TRANSPOSED: the key-pad
                                   mask is then a per-partition bias)
    expT[k,q] = exp(scT/sqrt(E) + mask_bias[k])   (ACT, psum->sbuf)
    p[k,e]    = transpose(pT)     (24 PE transposes)
    r[1,q]    = ones^T @ expT     (4 MMs)
    uT[e,q]   = p^T chunks @ expT (24 MMs); ot = ACT copy psum->sbuf
    y        += (1/r) * oT^T chunks @ Wo_h chunks (48 MMs, accum in SBUF)
  LayerNorm(y) * gamma + beta  -> out
"""

import math
from contextlib import ExitStack

import ml_dtypes
import numpy as np

B, S, E, H = 8, 512, 768, 12
EC = E // 128  # 6 chunks of e
SC = S // 128  # 4 chunks of s
FH = 2  # f halves of 384 for y matmuls
FW = E // FH  # 384
EPS = 1e-5
NEG = -1.0e30
INV_SQRT_E = 1.0 / math.sqrt(E)

_CACHE = {}


def _emit(nc, tc, tensors):
    import concourse.mybir as mybir

    F32 = mybir.dt.float32
    F32R = mybir.dt.float32r
    BF16 = mybir.dt.bfloat16
    I32 = mybir.dt.int32
    AF = mybir.ActivationFunctionType
    OP = mybir.AluOpType

    x_d, mask_d, wh_d, bh_d, wo_d, bo_d, gamma_d, beta_d, y_d = tensors

    FP8 = mybir.dt.float8e4
    DR = mybir.MatmulPerfMode.DoubleRow

    ctx = ExitStack()
    pool = lambda name, bufs, **kw: ctx.enter_context(
        tc.tile_pool(name=name, bufs=bufs, **kw)
    )
    constp = pool("const", 1)
    xtp = pool("xt", 1)
    yp = pool("y", 1)
    ps_proj = pool("ps_proj", 2, space="PSUM")
    ps_sc = pool("ps_sc", 1, space="PSUM")
    ps_yr = pool("ps_yr", 2, space="PSUM")
    ps_scr = pool("ps_scr", 3, space="PSUM")

    # ---- constants ----
    ident_d = nc.inline_tensor(
        np.eye(128, dtype=ml_dtypes.bfloat16), name="ident128"
    )
    ident = constp.tile([128, 128], BF16)
    nc.gpsimd.dma_start(ident[:], ident_d.ap())
    ones_col_d = nc.inline_tensor(
        np.ones((128, 1), dtype=ml_dtypes.bfloat16), name="ones_col"
    )
    ones_col = constp.tile([128, 1], BF16)
    nc.gpsimd.dma_start(ones_col[:], ones_col_d.ap())
    ones_row_d = nc.inline_tensor(np.ones((1, 128), dtype=np.float32), name="ones_row")
    ones_row = constp.tile([1, 128], F32R)
    nc.gpsimd.dma_start(ones_row[:], ones_row_d.ap())
    eps_t = constp.tile([128, 1], F32)
    nc.vector.memset(eps_t[:], EPS)
    ident1 = constp.tile([1, 1], F32)
    nc.vector.memset(ident1[:], 1.0)

    mask_bias = constp.tile([128, SC], F32)
    bo_row = constp.tile([1, E], F32R)
    nc.sync.dma_start(bo_row[:], bo_d.ap())
    gamma_bc = constp.tile([128, E], F32)
    beta_bc = constp.tile([128, E], F32)
    bo_bc = constp.tile([128, E], F32)

    xt = xtp.tile([128, EC * S], BF16)
    y_sb = yp.tile([128, SC * E], F32)

    whp = pool("wh", 2)
    wop = pool("wo", 2)
    bhp = pool("bh", 2)
    ptp = pool("pt", 2)
    pp = pool("p", 2)
    expp = pool("expt", 2)
    otp = pool("ot", 2)
    smallp = pool("small", 2)
    statp = pool("stat", 10)
    lnp = pool("ln", 3)

    def load_wh(h):
        # Split per chunk so head-0's pT accumulation can start as soon as
        # the first chunk lands (the HWDGE queue completes in order).
        wh = whp.tile([128, EC * E], BF16, tag="wh")
        for ic in range(EC):
            nc.sync.dma_start(
                wh[:, ic * E : (ic + 1) * E],
                wh_d.ap()[h, ic * 128 : (ic + 1) * 128, :],
            )
        bh_t = bhp.tile([128, EC], F32, tag="bh")
        nc.sync.dma_start(bh_t[:], bh_d.ap()[h].rearrange("(c p) -> p c", p=128))
        return wh, bh_t

    def load_wo(h):
        wo = wop.tile([128, EC * E], BF16, tag="wo")
        nc.sync.dma_start(
            wo[:].rearrange("p (c e) -> p c e", c=EC),
            wo_d.ap()[h].rearrange("(c p) e -> p c e", p=128),
        )
        return wo

    # ---- setup: x slices first (the transposes and everything else need
    # them), then head-0 Wh chunks, then the rest. The first pT
    # accumulation can start as soon as wh0 lands (~x+wh0 bytes in).
    wh0 = whp.tile([128, EC * E], BF16, tag="wh")
    bh0 = bhp.tile([128, EC], F32, tag="bh")
    xall = ptp.tile([128, SC * E], BF16, tag="pt")
    xv = xall[:].rearrange("p (t e) -> p t e", t=SC)
    for ic in range(EC):
        nc.sync.dma_start(
            xv[:, :, ic * 128 : (ic + 1) * 128],
            x_d.ap()
            .rearrange("(t p) e -> p t e", p=128)[:, :, ic * 128 : (ic + 1) * 128],
        )
    for ic in range(EC):
        nc.sync.dma_start(
            wh0[:, ic * E : (ic + 1) * E],
            wh_d.ap()[0, ic * 128 : (ic + 1) * 128, :],
        )
    nc.sync.dma_start(bh0[:], bh_d.ap()[0].rearrange("(c p) -> p c", p=128))

    mask_i = statp.tile([128, SC], I32, tag="stat")
    nc.sync.dma_start(mask_i[:], mask_d.ap()[0].rearrange("(c p) -> p c", p=128))
    nc.vector.tensor_scalar(
        out=mask_bias[:], in0=mask_i[:], scalar1=NEG, scalar2=None, op0=OP.mult
    )

    # 4 transposes batched into one PSUM bank tile -> one copy per ec
    for ec in range(EC):
        trp = ps_scr.tile([128, 512], BF16, tag="scratch")
        for t in range(SC):
            nc.tensor.transpose(
                trp[:, t * 128 : (t + 1) * 128],
                xall[:, t * E + ec * 128 : t * E + ec * 128 + 128],
                ident[:],
            )
        nc.any.tensor_copy(xt[:, ec * S : (ec + 1) * S], trp[:])

    wo0 = load_wo(0)
    loaded0 = (wh0, bh0, wo0)

    gamma_row = lnp.tile([1, E], F32R, tag="lnt")
    nc.sync.dma_start(gamma_row[:], gamma_d.ap())
    beta_row = lnp.tile([1, E], F32R, tag="lnsq")
    nc.sync.dma_start(beta_row[:], beta_d.ap())

    def emit_bcasts():
        # broadcast gamma/beta/bo rows to all 128 partitions via K=1
        # matmuls; emitted after head-0's pT so these PE ops (which wait
        # on the gamma/beta DMAs at the tail of the load queue) don't
        # block head-0's first matmuls in the in-order PE queue.
        for row, bc in ((gamma_row, gamma_bc), (beta_row, beta_bc), (bo_row, bo_bc)):
            for f in range(FH):
                bps = ps_scr.tile([128, FW], F32, tag="scratch")
                nc.tensor.matmul(
                    bps[:],
                    ones_row[:],
                    row[:, f * FW : (f + 1) * FW],
                    start=True,
                    stop=True,
                )
                nc.any.tensor_copy(bc[:, f * FW : (f + 1) * FW], bps[:])

    def layernorm(qt):
        # Engine split keeps the DVE queue clear of big LN ops while it
        # still has head-11 psum y-drains to run: stats via ACT accum_out
        # passes, (y-mu)*gamma on GpSimd, final *rstd+beta as ONE DVE
        # affine_then_add. For the last qt (nothing left on DVE) the DVE
        # bn_stats path is shorter.
        ys = y_sb[:, qt * E : (qt + 1) * E]
        last = qt == SC - 1
        mu = statp.tile([128, 1], F32, tag="stat")
        var = statp.tile([128, 1], F32, tag="stat")
        if last:
            stats = statp.tile([128, 12], F32, tag="stat")
            for c in range(2):
                nc.vector.bn_stats(
                    stats[:, c * 6 : (c + 1) * 6],
                    y_sb[:, qt * E + c * FW : qt * E + (c + 1) * FW],
                )
            mv = statp.tile([128, 2], F32, tag="stat")
            nc.vector.bn_aggr(mv[:], stats[:])
            mu, var = mv[:, 0:1], mv[:, 1:2]
        else:
            scr = lnp.tile([128, E], BF16, tag="lnscr")
            musum = statp.tile([128, 1], F32, tag="stat")
            nc.scalar.activation(scr[:], ys, AF.Copy, accum_out=musum[:])
            scr2 = lnp.tile([128, E], BF16, tag="lnscr2")
            ssq = statp.tile([128, 1], F32, tag="stat")
            nc.scalar.activation(scr2[:], ys, AF.Square, accum_out=ssq[:])
            nc.scalar.activation(mu[:], musum[:], AF.Copy, scale=1.0 / E)
            m2 = statp.tile([128, 1], F32, tag="stat")
            nc.vector.tensor_tensor(out=m2[:], in0=mu[:], in1=mu[:], op=OP.mult)
            nc.vector.scalar_tensor_tensor(
                out=var[:],
                in0=ssq[:],
                scalar=1.0 / E,
                in1=m2[:],
                op0=OP.mult,
                op1=OP.subtract,
            )
            mu, var = mu[:], var[:]
        std = statp.tile([128, 1], F32, tag="stat")
        nc.scalar.activation(std[:], var, AF.Sqrt, bias=eps_t[:], scale=1.0)
        rstd = statp.tile([128, 1], F32, tag="stat")
        nc.vector.reciprocal(rstd[:], std[:])
        # (gpsimd can't run TensorScalarPtr ops - codegen rejects them)
        o1 = lnp.tile([128, E], F32, tag="lnt")
        nc.vector.scalar_tensor_tensor(
            out=o1[:],
            in0=ys,
            scalar=mu,
            in1=gamma_bc[:],
            op0=OP.subtract,
            op1=OP.mult,
        )
        yout = lnp.tile([128, E], F32, tag="lnsq")
        nc.vector.affine_then_add(
            out=yout[:], in0=o1[:], in1=beta_bc[:], scale=rstd[:], bias=0.0
        )
        nc.sync.dma_start(y_d.ap()[qt * 128 : (qt + 1) * 128, :], yout[:])

    def head(h, loaded, post_pt=None):
        wh, bh_t, wo = loaded

        # pT[e,s]: chunk et <- sum_ic Wh[ic, et]^T @ xT[ic]  (+bh)
        # NOTE: fp8-DoubleRow scores were tried and REJECTED: rows whose
        # own key is masked get o = softmax-average with ~10x smaller
        # variance, and the final LayerNorm divides by that row's std,
        # amplifying injected score noise ~10x (measured 3.5e-2 rel err
        # vs the 2e-2 gate). bf16 everywhere it is.
        pt = ptp.tile([128, EC * S], BF16, tag="pt")
        for et in range(EC):
            pps = ps_proj.tile([128, S], F32, tag="proj")
            for ic in range(EC):
                nc.tensor.matmul(
                    pps[:],
                    wh[:, ic * E + et * 128 : ic * E + et * 128 + 128],
                    xt[:, ic * S : (ic + 1) * S],
                    start=(ic == 0),
                    stop=(ic == EC - 1),
                )
            nc.scalar.activation(
                pt[:, et * S : (et + 1) * S],
                pps[:],
                AF.Identity,
                bias=bh_t[:, et : et + 1],
                scale=1.0,
            )
        if post_pt is not None:
            post_pt()

        # scoresT[k,q]: the pre-mask score matrix is symmetric
        # (s[q,k] = p_q . p_k), so compute only the blocks qt >= kt, drain
        # raw scores to SBUF (bf16), and fill each row's lower blocks by
        # PE-transposing the already-drained upper blocks (128-cycle
        # transposes instead of 768-cycle K-accumulated matmuls). exp then
        # reads the assembled SBUF row with the per-partition key mask.
        # p[k,e] = transpose(pT) is interleaved per kt in small batched
        # bursts so the PE never sits in a long transpose-only phase
        # (HAM doesn't count transpose-mode as busy -> re-throttle).
        sc_sb = scp.tile([128, SC * S], BF16)
        expt = expp.tile([128, SC * S], BF16)
        p_t = pp.tile([128, SC * E], BF16)
        for kt in range(SC):
            w = (SC - kt) * 128
            scs = ps_sc.tile([128, S], F32, tag="sc")
            for ec in range(EC):
                nc.tensor.matmul(
                    scs[:, :w],
                    pt[:, ec * S + kt * 128 : ec * S + kt * 128 + 128],
                    pt[:, ec * S + kt * 128 : (ec + 1) * S],
                    start=(ec == 0),
                    stop=(ec == EC - 1),
                )
            nc.scalar.copy(
                sc_sb[:, kt * S + kt * 128 : (kt + 1) * S], scs[:, :w]
            )
            if kt:
                trp = ps_scr.tile([128, 512], BF16, tag="scratch")
                for qt in range(kt):
                    nc.tensor.transpose(
                        trp[:, qt * 128 : (qt + 1) * 128],
                        sc_sb[:, qt * S + kt * 128 : qt * S + (kt + 1) * 128],
                        ident[:],
                    )
                nc.any.tensor_copy(
                    sc_sb[:, kt * S : kt * S + kt * 128], trp[:, : kt * 128]
                )
            nc.scalar.activation(
                expt[:, kt * S : (kt + 1) * S],
                sc_sb[:, kt * S : (kt + 1) * S],
                AF.Exp,
                bias=mask_bias[:, kt : kt + 1],
                scale=INV_SQRT_E,
            )
            for g, gw in ((0, 4), (4, 2)):
                trp = ps_scr.tile([128, 512], BF16, tag="scratch")
                for j in range(gw):
                    ec = g + j
                    nc.tensor.transpose(
                        trp[:, j * 128 : (j + 1) * 128],
                        pt[:, ec * S + kt * 128 : ec * S + kt * 128 + 128],
                        ident[:],
                    )
                nc.any.tensor_copy(
                    p_t[:, kt * E + g * 128 : kt * E + (g + gw) * 128],
                    trp[:, : gw * 128],
                )

        # r[1,q] = sum_k expT  (emitted before the transposes so the DVE
        # reciprocal below overlaps the PE transpose/uT stream)
        rps = ps_yr.tile([1, S], F32, tag="yr")
        for kt in range(SC):
            nc.tensor.matmul(
                rps[:],
                ones_col[:],
                expt[:, kt * S : (kt + 1) * S],
                start=(kt == 0),
                stop=(kt == SC - 1),
            )
        # transpose r into per-partition layout [128, SC] (tiny PE
        # transposes), then one fast [128,4] reciprocal. 1/r is applied
        # per-partition at the y accumulation, so no broadcast is needed.
        r_sb = smallp.tile([1, S], F32, tag="rsb")
        nc.scalar.copy(r_sb[:], rps[:])
        rtp = ps_sc.tile([128, SC], F32, tag="sc")
        for qt in range(SC):
            nc.tensor.transpose(
                rtp[:, qt : qt + 1],
                r_sb[:, qt * 128 : (qt + 1) * 128],
                ident1[:],
            )
        rsum_t = smallp.tile([128, SC], F32, tag="rsum")
        nc.scalar.copy(rsum_t[:], rtp[:])
        recip_col = smallp.tile([128, SC], F32, tag="recipc")
        nc.vector.reciprocal_approx_fast(recip_col[:], rsum_t[:])

        # uT[e,q] = sum_k p[k,e]^T @ expT[k]; drain to bf16 for the y MMs.
        ot = otp.tile([128, EC * S], BF16)
        for et in range(EC):
            ups = ps_proj.tile([128, S], F32, tag="proj")
            for kt in range(SC):
                nc.tensor.matmul(
                    ups[:],
                    p_t[:, kt * E + et * 128 : kt * E + et * 128 + 128],
                    expt[:, kt * S : (kt + 1) * S],
                    start=(kt == 0),
                    stop=(kt == SC - 1),
                )
            nc.scalar.copy(ot[:, et * S : (et + 1) * S], ups[:])

        # y[q,f] += (1/r[q]) * sum_e uT[e,q]^T @ Wo_h[e,f]   (+bo on head 0)
        # 1/r is a per-partition scalar in the q-partitioned y tiles.
        for qt in range(SC):
            rc = recip_col[:, qt : qt + 1]
            for f in range(FH):
                yps = ps_yr.tile([128, FW], F32, tag="yr")
                for ec in range(EC):
                    nc.tensor.matmul(
                        yps[:],
                        ot[:, ec * S + qt * 128 : ec * S + qt * 128 + 128],
                        wo[:, ec * E + f * FW : ec * E + (f + 1) * FW],
                        start=(ec == 0),
                        stop=(ec == EC - 1),
                    )
                ysl = y_sb[:, qt * E + f * FW : qt * E + (f + 1) * FW]
                other = bo_bc[:, f * FW : (f + 1) * FW] if h == 0 else ysl
                nc.vector.scalar_tensor_tensor(
                    out=ysl,
                    in0=yps[:],
                    scalar=rc,
                    in1=other,
                    op0=OP.mult,
                    op1=OP.add,
                )
            # last head: emit LayerNorm for this qt right away so it
            # overlaps the remaining y matmuls on ACT/DVE/DMA.
            if h == H - 1:
                layernorm(qt)

    loaded = loaded0
    for h in range(H):
        with nc.named_scope(f"head{h}"):
            if h + 1 < H:
                nwh, nbh = load_wh(h + 1)
                nxt = (nwh, nbh, load_wo(h + 1))
            else:
                nxt = None
            head(h, loaded, post_pt=emit_bcasts if h == 0 else None)
            loaded = nxt

    ctx.close()


def _build_nc():
    import concourse.bacc as bacc
    import concourse.mybir as mybir
    import concourse.tile as tile

    F32 = mybir.dt.float32
    BF16 = mybir.dt.bfloat16
    I32 = mybir.dt.int32

    nc = bacc.Bacc("TRN2", target_bir_lowering=False, debug=False, enable_asserts=True)

    F32R = mybir.dt.float32r
    tensors = (
        nc.dram_tensor("x", [S, E], BF16, kind="ExternalInput"),
        nc.dram_tensor("mask", [1, S], I32, kind="ExternalInput"),
        nc.dram_tensor("wh", [H, E, E], BF16, kind="ExternalInput"),
        nc.dram_tensor("bh", [H, E], F32, kind="ExternalInput"),
        nc.dram_tensor("wo", [H, E, E], BF16, kind="ExternalInput"),
        nc.dram_tensor("bo", [1, E], F32R, kind="ExternalInput"),
        nc.dram_tensor("gamma", [1, E], F32R, kind="ExternalInput"),
        nc.dram_tensor("beta", [1, E], F32R, kind="ExternalInput"),
        nc.dram_tensor("y", [S, E], F32, kind="ExternalOutput"),
    )

    with tile.TileContext(nc) as tc:
        _emit(nc, tc, tensors)

    nc.compile()
    return nc


def get_nc():
    if "nc" not in _CACHE:
        _CACHE["nc"] = _build_nc()
    return _CACHE["nc"]


def make_in_maps(x, atten_pad_mask, Wh, bh, Wo, bo, gamma, beta):
    bf16 = ml_dtypes.bfloat16
    x = np.ascontiguousarray(np.asarray(x, dtype=np.float32).astype(bf16))
    mask = np.ascontiguousarray(np.asarray(atten_pad_mask, dtype=np.int32))
    wh = np.ascontiguousarray(np.asarray(Wh, dtype=np.float32).astype(bf16))
    bhv = np.ascontiguousarray(np.asarray(bh, dtype=np.float32))
    wo = np.ascontiguousarray(
        np.asarray(Wo, dtype=np.float32).astype(bf16).reshape(H, E, E)
    )
    bov = np.asarray(bo, dtype=np.float32).reshape(1, E)
    gam = np.asarray(gamma, dtype=np.float32).reshape(1, E)
    bet = np.asarray(beta, dtype=np.float32).reshape(1, E)
    return [
        {
            "x": x[b],
            "mask": mask[b],
            "wh": wh,
            "bh": bhv,
            "wo": wo,
            "bo": bov,
            "gamma": gam,
            "beta": bet,
        }
        for b in range(B)
    ]


def kernel(x, atten_pad_mask, Wh, bh, Wo, bo, gamma, beta):
    from concourse.bass_utils import run_bass_kernel_spmd

    nc = get_nc()
    in_maps = make_in_maps(x, atten_pad_mask, Wh, bh, Wo, bo, gamma, beta)
    res = run_bass_kernel_spmd(nc, in_maps, list(range(B)))
    return np.stack([res.results[b]["y"] for b in range(B)], axis=0)


# revision 22
# speedup vs baseline: 1.0756x; 1.0756x over previous
"""Multi-head self-attention (shared q/k/v projection per head) + output
projection + LayerNorm, data-parallel over batch across 8 NeuronCores.

Shapes (hardcoded): B=8, S=512, E=768, H=12.
Each core handles one batch element b: full attention for all 12 heads,
the output projection, and the final LayerNorm. No collectives; the host
scatters x/mask per batch element and concatenates the 8 outputs.

v2: all big matmul operands in bf16 (weights converted host-side).
  - bf16 enables Fast Weight Load (LDWEIGHTS at 2x; fp32r disables FWL),
    halves weight DMA (56->28 MB/core), and makes PE transposes 1.0
    cycles/row instead of 1.5.
  - PSUM accumulation stays fp32; y accumulation across heads in SBUF
    fp32; LayerNorm in fp32. Measured rel err ~2e-3 (tolerance 2e-2).
  - LayerNorm uses DVE bn_stats/bn_aggr (one pass for mean+var) and is
    emitted inside head 11's y loop so it overlaps the last head's PE
    work instead of running as a serial tail.

Per-core dataflow:
  xT  = x^T                       (24 PE transposes, once)
  per head h:
    pT[e,s]   = Wh_h^T @ xT  + bh (36 MMs; bias applied in ACT psum->sbuf copy)
    scT[k,q]  = pT^T chunks @ pT  (24 MMs, scores TRANSPOSED: the key-pad
                                   mask is then a per-partition bias)
    expT[k,q] = exp(scT/sqrt(E) + mask_bias[k])   (ACT, psum->sbuf)
    p[k,e]    = transpose(pT)     (24 PE transposes)
    r[1,q]    = ones^T @ expT     (4 MMs)
    uT[e,q]   = p^T chunks @ expT (24 MMs); ot = ACT copy psum->sbuf
    y        += (1/r) * oT^T chunks @ Wo_h chunks (48 MMs, accum in SBUF)
  LayerNorm(y) * gamma + beta  -> out
"""

import math
from contextlib import ExitStack

import ml_dtypes
import numpy as np

B, S, E, H = 8, 512, 768, 12
EC = E // 128  # 6 chunks of e
SC = S // 128  # 4 chunks of s
FH = 2  # f halves of 384 for y matmuls
FW = E // FH  # 384
EPS = 1e-5
NEG = -1.0e30
INV_SQRT_E = 1.0 / math.sqrt(E)

_CACHE = {}


def _emit(nc, tc, tensors):
    import concourse.mybir as mybir

    F32 = mybir.dt.float32
    F32R = mybir.dt.float32r
    BF16 = mybir.dt.bfloat16
    I32 = mybir.dt.int32
    AF = mybir.ActivationFunctionType
    OP = mybir.AluOpType

    x_d, mask_d, wh_d, bh_d, wo_d, bo_d, gamma_d, beta_d, y_d = tensors

    FP8 = mybir.dt.float8e4
    DR = mybir.MatmulPerfMode.DoubleRow

    ctx = ExitStack()
    pool = lambda name, bufs, **kw: ctx.enter_context(
        tc.tile_pool(name=name, bufs=bufs, **kw)
    )
    constp = pool("const", 1)
    xtp = pool("xt", 1)
    yp = pool("y", 1)
    ps_proj = pool("ps_proj", 2, space="PSUM")
    ps_sc = pool("ps_sc", 2, space="PSUM")
    ps_yr = pool("ps_yr", 2, space="PSUM")
    ps_scr = pool("ps_scr", 2, space="PSUM")

    # ---- constants ----
    ident_d = nc.inline_tensor(
        np.eye(128, dtype=ml_dtypes.bfloat16), name="ident128"
    )
    ident = constp.tile([128, 128], BF16)
    nc.gpsimd.dma_start(ident[:], ident_d.ap())
    ones_col_d = nc.inline_tensor(
        np.ones((128, 1), dtype=ml_dtypes.bfloat16), name="ones_col"
    )
    ones_col = constp.tile([128, 1], BF16)
    nc.gpsimd.dma_start(ones_col[:], ones_col_d.ap())
    ones_row_d = nc.inline_tensor(np.ones((1, 128), dtype=np.float32), name="ones_row")
    ones_row = constp.tile([1, 128], F32R)
    nc.gpsimd.dma_start(ones_row[:], ones_row_d.ap())
    eps_t = constp.tile([128, 1], F32)
    nc.vector.memset(eps_t[:], EPS)
    ident1 = constp.tile([1, 1], F32)
    nc.vector.memset(ident1[:], 1.0)

    mask_bias = constp.tile([128, SC], F32)
    bo_row = constp.tile([1, E], F32R)
    nc.sync.dma_start(bo_row[:], bo_d.ap())
    gamma_bc = constp.tile([128, E], F32)
    beta_bc = constp.tile([128, E], F32)
    bo_bc = constp.tile([128, E], F32)

    xt = xtp.tile([128, EC * S], BF16)
    y_sb = yp.tile([128, SC * E], F32)

    whp = pool("wh", 2)
    wop = pool("wo", 2)
    bhp = pool("bh", 2)
    ptp = pool("pt", 2)
    scp = pool("scsb", 2)
    pp = pool("p", 2)
    expp = pool("expt", 2)
    otp = pool("ot", 2)
    smallp = pool("small", 2)
    statp = pool("stat", 24)
    lnp = pool("ln", 3)

    def load_wh(h):
        # Split per chunk so head-0's pT accumulation can start as soon as
        # the first chunk lands (the HWDGE queue completes in order).
        wh = whp.tile([128, EC * E], BF16, tag="wh")
        for ic in range(EC):
            nc.sync.dma_start(
                wh[:, ic * E : (ic + 1) * E],
                wh_d.ap()[h, ic * 128 : (ic + 1) * 128, :],
            )
        bh_t = bhp.tile([128, EC], F32, tag="bh")
        nc.sync.dma_start(bh_t[:], bh_d.ap()[h].rearrange("(c p) -> p c", p=128))
        return wh, bh_t

    def load_wo(h):
        wo = wop.tile([128, EC * E], BF16, tag="wo")
        nc.sync.dma_start(
            wo[:].rearrange("p (c e) -> p c e", c=EC),
            wo_d.ap()[h].rearrange("(c p) e -> p c e", p=128),
        )
        return wo

    # ---- setup: x slices on the sync queue; head-0 Wh/bh on the Scalar
    # DGE queue so x and wh0 stream in parallel and head-0's pT can start
    # after ~max(x, wh0) bytes instead of their sum.
    wh0 = whp.tile([128, EC * E], BF16, tag="wh")
    bh0 = bhp.tile([128, EC], F32, tag="bh")
    xall = ptp.tile([128, SC * E], BF16, tag="pt")
    xv = xall[:].rearrange("p (t e) -> p t e", t=SC)
    for ic in range(EC):
        nc.sync.dma_start(
            xv[:, :, ic * 128 : (ic + 1) * 128],
            x_d.ap()
            .rearrange("(t p) e -> p t e", p=128)[:, :, ic * 128 : (ic + 1) * 128],
        )
    for ic in range(EC):
        nc.scalar.dma_start(
            wh0[:, ic * E : (ic + 1) * E],
            wh_d.ap()[0, ic * 128 : (ic + 1) * 128, :],
        )
    nc.scalar.dma_start(bh0[:], bh_d.ap()[0].rearrange("(c p) -> p c", p=128))

    mask_i = statp.tile([128, SC], I32, tag="stat")
    nc.sync.dma_start(mask_i[:], mask_d.ap()[0].rearrange("(c p) -> p c", p=128))
    nc.vector.tensor_scalar(
        out=mask_bias[:], in0=mask_i[:], scalar1=NEG, scalar2=None, op0=OP.mult
    )

    # 4 transposes batched into one PSUM bank tile -> one copy per ec
    for ec in range(EC):
        trp = ps_scr.tile([128, 512], BF16, tag="scratch")
        for t in range(SC):
            nc.tensor.transpose(
                trp[:, t * 128 : (t + 1) * 128],
                xall[:, t * E + ec * 128 : t * E + ec * 128 + 128],
                ident[:],
            )
        nc.any.tensor_copy(xt[:, ec * S : (ec + 1) * S], trp[:])

    gamma_row = lnp.tile([1, E], F32R, tag="lngr")
    nc.sync.dma_start(gamma_row[:], gamma_d.ap())
    beta_row = lnp.tile([1, E], F32R, tag="lnbr")
    nc.sync.dma_start(beta_row[:], beta_d.ap())

    wo0 = load_wo(0)
    loaded0 = (wh0, bh0, wo0)

    def emit_bcasts():
        # broadcast gamma/beta/bo rows to all 128 partitions via K=1
        # matmuls; emitted after head-0's pT so these PE ops (which wait
        # on the gamma/beta DMAs at the tail of the load queue) don't
        # block head-0's first matmuls in the in-order PE queue.
        for row, bc in ((gamma_row, gamma_bc), (beta_row, beta_bc), (bo_row, bo_bc)):
            for f in range(FH):
                bps = ps_scr.tile([128, FW], F32, tag="scratch")
                nc.tensor.matmul(
                    bps[:],
                    ones_row[:],
                    row[:, f * FW : (f + 1) * FW],
                    start=True,
                    stop=True,
                )
                nc.any.tensor_copy(bc[:, f * FW : (f + 1) * FW], bps[:])

    # LayerNorm, split into three emission phases that are interleaved
    # one y-quarter apart inside head 11 so no engine FIFO ever blocks on
    # a cross-engine dependency: stats on ACT (accum_out passes), the
    # tiny mean/var chain on DVE, sqrt back on ACT (emitted a quarter
    # later, when var is long done), then recip + the two fused
    # normalization ops on DVE. (gpsimd can't run TensorScalarPtr ops -
    # codegen rejects them - so the big ops stay on DVE.)
    ln = {}

    def ln_stats(qt):
        ys = y_sb[:, qt * E : (qt + 1) * E]
        scr = lnp.tile([128, E], BF16, tag="lnscr")
        musum = statp.tile([128, 1], F32, tag="stat")
        nc.scalar.activation(scr[:], ys, AF.Copy, accum_out=musum[:])
        scr2 = lnp.tile([128, E], BF16, tag="lnscr2")
        ssq = statp.tile([128, 1], F32, tag="stat")
        nc.scalar.activation(scr2[:], ys, AF.Square, accum_out=ssq[:])
        ln[qt] = {"musum": musum, "ssq": ssq}

    def ln_mid(qt):
        s = ln[qt]
        mu = statp.tile([128, 1], F32, tag="stat")
        nc.vector.tensor_scalar(
            out=mu[:], in0=s["musum"][:], scalar1=1.0 / E, scalar2=None, op0=OP.mult
        )
        m2 = statp.tile([128, 1], F32, tag="stat")
        nc.vector.tensor_tensor(out=m2[:], in0=mu[:], in1=mu[:], op=OP.mult)
        var = statp.tile([128, 1], F32, tag="stat")
        nc.vector.scalar_tensor_tensor(
            out=var[:],
            in0=s["ssq"][:],
            scalar=1.0 / E,
            in1=m2[:],
            op0=OP.mult,
            op1=OP.subtract,
        )
        s["mu"], s["var"] = mu, var

    def ln_fin(qt):
        s = ln[qt]
        ys = y_sb[:, qt * E : (qt + 1) * E]
        std = statp.tile([128, 1], F32, tag="stat")
        nc.scalar.activation(std[:], s["var"][:], AF.Sqrt, bias=eps_t[:], scale=1.0)
        rstd = statp.tile([128, 1], F32, tag="stat")
        nc.vector.reciprocal(rstd[:], std[:])
        o1 = lnp.tile([128, E], F32, tag="lnt")
        nc.vector.scalar_tensor_tensor(
            out=o1[:],
            in0=ys,
            scalar=s["mu"][:],
            in1=gamma_bc[:],
            op0=OP.subtract,
            op1=OP.mult,
        )
        yout = lnp.tile([128, E], F32, tag="lnsq")
        nc.vector.affine_then_add(
            out=yout[:], in0=o1[:], in1=beta_bc[:], scale=rstd[:], bias=0.0
        )
        nc.sync.dma_start(y_d.ap()[qt * 128 : (qt + 1) * 128, :], yout[:])

    def head(h, loaded, post_pt=None):
        wh, bh_t, wo = loaded

        # pT[e,s]: chunk et <- sum_ic Wh[ic, et]^T @ xT[ic]  (+bh)
        # NOTE: fp8-DoubleRow scores were tried and REJECTED: rows whose
        # own key is masked get o = softmax-average with ~10x smaller
        # variance, and the final LayerNorm divides by that row's std,
        # amplifying injected score noise ~10x (measured 3.5e-2 rel err
        # vs the 2e-2 gate). bf16 everywhere it is.
        pt = ptp.tile([128, EC * S], BF16, tag="pt")
        for et in range(EC):
            pps = ps_proj.tile([128, S], F32, tag="proj")
            for ic in range(EC):
                nc.tensor.matmul(
                    pps[:],
                    wh[:, ic * E + et * 128 : ic * E + et * 128 + 128],
                    xt[:, ic * S : (ic + 1) * S],
                    start=(ic == 0),
                    stop=(ic == EC - 1),
                )
            nc.scalar.activation(
                pt[:, et * S : (et + 1) * S],
                pps[:],
                AF.Identity,
                bias=bh_t[:, et : et + 1],
                scale=1.0,
            )
        if post_pt is not None:
            post_pt()

        # scoresT[k,q]: the pre-mask score matrix is symmetric
        # (s[q,k] = p_q . p_k), so compute only the blocks qt >= kt, drain
        # raw scores to SBUF (bf16), and fill each row's lower blocks by
        # PE-transposing the already-drained upper blocks (128-cycle
        # transposes instead of 768-cycle K-accumulated matmuls). exp then
        # reads the assembled SBUF row with the per-partition key mask.
        # p[k,e] = transpose(pT) is interleaved per kt in small batched
        # bursts so the PE never sits in a long transpose-only phase
        # (HAM doesn't count transpose-mode as busy -> re-throttle).
        sc_sb = scp.tile([128, SC * S], BF16)
        expt = expp.tile([128, SC * S], BF16)
        p_t = pp.tile([128, SC * E], BF16)
        for kt in range(SC):
            w = (SC - kt) * 128
            scs = ps_sc.tile([128, S], F32, tag="sc")
            for ec in range(EC):
                nc.tensor.matmul(
                    scs[:, :w],
                    pt[:, ec * S + kt * 128 : ec * S + kt * 128 + 128],
                    pt[:, ec * S + kt * 128 : (ec + 1) * S],
                    start=(ec == 0),
                    stop=(ec == EC - 1),
                )
            nc.scalar.copy(
                sc_sb[:, kt * S + kt * 128 : (kt + 1) * S], scs[:, :w]
            )
            if kt:
                trp = ps_scr.tile([128, 512], BF16, tag="scratch")
                for qt in range(kt):
                    nc.tensor.transpose(
                        trp[:, qt * 128 : (qt + 1) * 128],
                        sc_sb[:, qt * S + kt * 128 : qt * S + (kt + 1) * 128],
                        ident[:],
                    )
                nc.any.tensor_copy(
                    sc_sb[:, kt * S : kt * S + kt * 128], trp[:, : kt * 128]
                )
            nc.scalar.activation(
                expt[:, kt * S : (kt + 1) * S],
                sc_sb[:, kt * S : (kt + 1) * S],
                AF.Exp,
                bias=mask_bias[:, kt : kt + 1],
                scale=INV_SQRT_E,
            )
            for g, gw in ((0, 4), (4, 2)):
                trp = ps_scr.tile([128, 512], BF16, tag="scratch")
                for j in range(gw):
                    ec = g + j
                    nc.tensor.transpose(
                        trp[:, j * 128 : (j + 1) * 128],
                        pt[:, ec * S + kt * 128 : ec * S + kt * 128 + 128],
                        ident[:],
                    )
                nc.any.tensor_copy(
                    p_t[:, kt * E + g * 128 : kt * E + (g + gw) * 128],
                    trp[:, : gw * 128],
                )

        # r[1,q] = sum_k expT  (emitted before the transposes so the DVE
        # reciprocal below overlaps the PE transpose/uT stream)
        rps = ps_yr.tile([1, S], F32, tag="yr")
        for kt in range(SC):
            nc.tensor.matmul(
                rps[:],
                ones_col[:],
                expt[:, kt * S : (kt + 1) * S],
                start=(kt == 0),
                stop=(kt == SC - 1),
            )
        # transpose r into per-partition layout [128, SC] (tiny PE
        # transposes), then one fast [128,4] reciprocal. 1/r is applied
        # per-partition at the y accumulation, so no broadcast is needed.
        r_sb = smallp.tile([1, S], F32, tag="rsb")
        nc.scalar.copy(r_sb[:], rps[:])
        rtp = ps_sc.tile([128, SC], F32, tag="sc")
        for qt in range(SC):
            nc.tensor.transpose(
                rtp[:, qt : qt + 1],
                r_sb[:, qt * 128 : (qt + 1) * 128],
                ident1[:],
            )
        rsum_t = smallp.tile([128, SC], F32, tag="rsum")
        nc.scalar.copy(rsum_t[:], rtp[:])
        recip_col = smallp.tile([128, SC], F32, tag="recipc")
        nc.vector.reciprocal_approx_fast(recip_col[:], rsum_t[:])

        # uT[e,q] = sum_k p[k,e]^T @ expT[k]; drain to bf16 for the y MMs.
        ot = otp.tile([128, EC * S], BF16)
        for et in range(EC):
            ups = ps_proj.tile([128, S], F32, tag="proj")
            for kt in range(SC):
                nc.tensor.matmul(
                    ups[:],
                    p_t[:, kt * E + et * 128 : kt * E + et * 128 + 128],
                    expt[:, kt * S : (kt + 1) * S],
                    start=(kt == 0),
                    stop=(kt == SC - 1),
                )
            nc.scalar.copy(ot[:, et * S : (et + 1) * S], ups[:])

        # y[q,f] += (1/r[q]) * sum_e uT[e,q]^T @ Wo_h[e,f]   (+bo on head 0)
        # 1/r is a per-partition scalar in the q-partitioned y tiles.
        for qt in range(SC):
            rc = recip_col[:, qt : qt + 1]
            for f in range(FH):
                yps = ps_yr.tile([128, FW], F32, tag="yr")
                for ec in range(EC):
                    nc.tensor.matmul(
                        yps[:],
                        ot[:, ec * S + qt * 128 : ec * S + qt * 128 + 128],
                        wo[:, ec * E + f * FW : ec * E + (f + 1) * FW],
                        start=(ec == 0),
                        stop=(ec == EC - 1),
                    )
                ysl = y_sb[:, qt * E + f * FW : qt * E + (f + 1) * FW]
                other = bo_bc[:, f * FW : (f + 1) * FW] if h == 0 else ysl
                nc.vector.scalar_tensor_tensor(
                    out=ysl,
                    in0=yps[:],
                    scalar=rc,
                    in1=other,
                    op0=OP.mult,
                    op1=OP.add,
                )
            # last head: emit the LayerNorm phases staggered one y-quarter
            # apart so they overlap the remaining y matmuls without any
            # engine FIFO blocking on a cross-engine dependency.
            if h == H - 1:
                ln_stats(qt)
                if qt >= 1:
                    ln_mid(qt - 1)
                if qt >= 2:
                    ln_fin(qt - 2)
        if h == H - 1:
            ln_mid(SC - 1)
            ln_fin(SC - 2)
            ln_fin(SC - 1)

    loaded = loaded0
    for h in range(H):
        with nc.named_scope(f"head{h}"):
            if h + 1 < H:
                nwh, nbh = load_wh(h + 1)
                nxt = (nwh, nbh, load_wo(h + 1))
            else:
                nxt = None
            head(h, loaded, post_pt=emit_bcasts if h == 0 else None)
            loaded = nxt

    ctx.close()


def _build_nc():
    import concourse.bacc as bacc
    import concourse.mybir as mybir
    import concourse.tile as tile

    F32 = mybir.dt.float32
    BF16 = mybir.dt.bfloat16
    I32 = mybir.dt.int32

    nc = bacc.Bacc("TRN2", target_bir_lowering=False, debug=False, enable_asserts=True)

    F32R = mybir.dt.float32r
    tensors = (
        nc.dram_tensor("x", [S, E], BF16, kind="ExternalInput"),
        nc.dram_tensor("mask", [1, S], I32, kind="ExternalInput"),
        nc.dram_tensor("wh", [H, E, E], BF16, kind="ExternalInput"),
        nc.dram_tensor("bh", [H, E], F32, kind="ExternalInput"),
        nc.dram_tensor("wo", [H, E, E], BF16, kind="ExternalInput"),
        nc.dram_tensor("bo", [1, E], F32R, kind="ExternalInput"),
        nc.dram_tensor("gamma", [1, E], F32R, kind="ExternalInput"),
        nc.dram_tensor("beta", [1, E], F32R, kind="ExternalInput"),
        nc.dram_tensor("y", [S, E], F32, kind="ExternalOutput"),
    )

    with tile.TileContext(nc) as tc:
        _emit(nc, tc, tensors)

    nc.compile()
    return nc


def get_nc():
    if "nc" not in _CACHE:
        _CACHE["nc"] = _build_nc()
    return _CACHE["nc"]


def make_in_maps(x, atten_pad_mask, Wh, bh, Wo, bo, gamma, beta):
    bf16 = ml_dtypes.bfloat16
    x = np.ascontiguousarray(np.asarray(x, dtype=np.float32).astype(bf16))
    mask = np.ascontiguousarray(np.asarray(atten_pad_mask, dtype=np.int32))
    wh = np.ascontiguousarray(np.asarray(Wh, dtype=np.float32).astype(bf16))
    bhv = np.ascontiguousarray(np.asarray(bh, dtype=np.float32))
    wo = np.ascontiguousarray(
        np.asarray(Wo, dtype=np.float32).astype(bf16).reshape(H, E, E)
    )
    bov = np.asarray(bo, dtype=np.float32).reshape(1, E)
    gam = np.asarray(gamma, dtype=np.float32).reshape(1, E)
    bet = np.asarray(beta, dtype=np.float32).reshape(1, E)
    return [
        {
            "x": x[b],
            "mask": mask[b],
            "wh": wh,
            "bh": bhv,
            "wo": wo,
            "bo": bov,
            "gamma": gam,
            "beta": bet,
        }
        for b in range(B)
    ]


def kernel(x, atten_pad_mask, Wh, bh, Wo, bo, gamma, beta):
    from concourse.bass_utils import run_bass_kernel_spmd

    nc = get_nc()
    in_maps = make_in_maps(x, atten_pad_mask, Wh, bh, Wo, bo, gamma, beta)
    res = run_bass_kernel_spmd(nc, in_maps, list(range(B)))
    return np.stack([res.results[b]["y"] for b in range(B)], axis=0)
